# revision 15
# baseline (speedup 1.0000x reference)
"""GAT-style kernel for Trainium2, 8 NeuronCores.

Math (per head, d = nhid):
    h  = xf @ W.T + b                  (N, d)
    h1 = h / max(||h||_row, eps)       row L2 normalize
    e  = h1 @ h1.T                     (N, N)  -- never materialized
    att = e / ||e||_col                column L2 normalize
    out = act(att @ h1)

Collapse: with G = h1.T @ h1 (d x d),
    ||e||_col[j]^2 = h1_j.T G h1_j     (row-wise quadratic form)
    att @ h1 = h1 @ M,  M = h1.T @ (h1 / cn)   (d x d)
so the N x N attention matrix is never formed and the whole computation
is O(N d^2).

Two modes:
  * "rep"  (default): every core gets the full input and computes the
    full output; no collectives.  On this stack AllReduce costs ~700us
    per call, far more than the whole collapsed computation, so
    replication wins despite 8x redundant compute.
  * "shard": row-shard N across the 8 cores, 4 small AllReduces
    (G, M for the hidden heads batched; G_o, M_o for the output head).
"""

import sys

for _p in ("/opt/trn_rl_repo", "/root/.axon_site/_ro/trn_rl_repo"):
    if _p not in sys.path:
        sys.path.append(_p)

import numpy as np

N_CORES = 8
N = 4096
NLOC = N // N_CORES          # 512 rows per core in shard mode
NFEAT = 128
NHID = 64
NCLASS = 16
EPS = 1e-12

_prog_cache = {}


def _patch_tile_drain():
    """Walrus in this container rejects Tile's tail drain (too many sync
    waits on one instruction).  Split it into one-wait-per-drain."""
    import concourse.tile as tile
    from concourse.vector_clock import ScopedClock, VectorClock

    if getattr(tile.TileContext, "_drain_split_patched", False):
        return

    def _drain_and_barrier(self, tick_clock, wait_clock):
        nc = self.nc
        gvc = tick_clock.global_clock  # VectorClock
        n = len(gvc)
        for proc in range(n):
            t = gvc[proc]
            if t > 0:
                sub = VectorClock([t if i == proc else 0 for i in range(n)])
                d = nc.sync.drain()
                wait_clock.add_sem_waits(d.ins, ScopedClock({None: sub}))
        nc.all_engine_barrier()
        assert self.sems is not None
        popped = nc._tile_sem_poison_stack.pop()
        assert popped is self._sem_poison
        nc.clear_and_free_semaphores(list(self.sems.allocated().values()))
        nc.all_engine_barrier()

    tile.TileContext._drain_and_barrier = _drain_and_barrier
    tile.TileContext._drain_split_patched = True


def _split_multi_waits(nc):
    """This container's walrus allows only one sync-wait per instruction.
    Hoist extra waits onto standalone same-engine NoOps."""
    import concourse.mybir as mybir

    n_new = 0
    for blk in nc.main_func.blocks:
        out = []
        changed = False
        for inst in blk.instructions:
            si = inst.sync_info
            waits = list(si.on_wait) if (si and si.on_wait) else []
            if len(waits) > 1:
                changed = True
                for w in waits[:-1]:
                    nop = mybir.InstNoOp(name=f"{inst.name}-xw{n_new}", ins=[], outs=[])
                    n_new += 1
                    nop.engine = inst.engine
                    nop.sync_info = mybir.SyncInfo(on_wait=[w], on_update=[])
                    out.append(nop)
                si.on_wait = [waits[-1]]
                inst.sync_info = si
            out.append(inst)
        if changed:
            blk.instructions = out


def _norm_scalars(nc, pool, q, name_tag):
    """q (128,1) sum of squares -> rn (128,1) = 1/max(sqrt(q), EPS)."""
    import concourse.mybir as mybir

    s = pool.tile([128, 1], mybir.dt.float32, tag=f"s_{name_tag}", name=f"s_{name_tag}")
    nc.scalar.sqrt(s[:], q[:])
    sm = pool.tile([128, 1], mybir.dt.float32, tag=f"sm_{name_tag}", name=f"sm_{name_tag}")
    nc.vector.tensor_scalar_max(sm[:], s[:], EPS)
    rn = pool.tile([128, 1], mybir.dt.float32, tag=f"rn_{name_tag}", name=f"rn_{name_tag}")
    nc.vector.reciprocal(rn[:], sm[:])
    return rn


def _allreduce(nc, dram_pool, sbuf_in, shape, tag):
    """AllReduce sbuf_in (shape) across all cores via DRAM bounce buffers."""
    import concourse.mybir as mybir

    bnc_in = dram_pool.tile(shape, mybir.dt.float32, name=f"arin_{tag}")
    bnc_out = dram_pool.tile(
        shape, mybir.dt.float32, addr_space="Shared", name=f"arout_{tag}"
    )
    nc.sync.dma_start(out=bnc_in[:], in_=sbuf_in[:])
    nc.gpsimd.collective_compute(
        "AllReduce",
        mybir.AluOpType.add,
        replica_groups=[list(range(N_CORES))],
        ins=[bnc_in.opt()],
        outs=[bnc_out.opt()],
    )
    return bnc_out


def _emit_body(nc, tc, ctx, tensors, rep, nloc, use_collectives, with_bias):
    import concourse.mybir as mybir
    from concourse.bass import ts
    from concourse.masks import make_identity

    f32 = mybir.dt.float32
    r = rep
    nt = nloc // 128        # row tiles
    nch = nloc // 512       # 512-wide column chunks in transposed layout

    x_d = tensors["xloc"]
    w123t_d = tensors["w123t"]
    b123_d = tensors["b123"]
    wot_hi_d = tensors["wot_hi"]
    wot_lo_d = tensors["wot_lo"]
    bo_d = tensors["bo"]
    out_d = tensors["outt"]

    const = ctx.enter_context(tc.tile_pool(name=f"const{r}", bufs=1))
    work = ctx.enter_context(tc.tile_pool(name=f"work{r}", bufs=2))
    small = ctx.enter_context(tc.tile_pool(name=f"small{r}", bufs=2))
    # PSUM budget is 8 banks; tags below sum to exactly 8:
    #   mm128 (2) + tr (1) + wide (1) + acc0/1/2 (3) + oacc (1)
    psum = ctx.enter_context(tc.tile_pool(name=f"psum{r}", bufs=1, space="PSUM"))
    dram = ctx.enter_context(tc.tile_pool(name=f"dram{r}", bufs=1, space="DRAM"))

    def ps_tile(shape, tag, name, bufs=1):
        return psum.tile(
            shape, mybir.dt.float32, tag=tag, name=f"{name}_{r}", bufs=bufs
        )

    # ---- constants / inputs to SBUF ----
    x_sb = const.tile([128, nloc], f32, name=f"x_sb{r}")
    nc.sync.dma_start(out=x_sb[:], in_=x_d[:])
    w123t_sb = const.tile([128, 3 * NHID], f32, name=f"w123t{r}")
    nc.sync.dma_start(out=w123t_sb[:], in_=w123t_d[:])
    wot_hi_sb = const.tile([128, NCLASS], f32, name=f"wot_hi{r}")
    nc.sync.dma_start(out=wot_hi_sb[:], in_=wot_hi_d[:])
    wot_lo_sb = const.tile([64, NCLASS], f32, name=f"wot_lo{r}")
    nc.sync.dma_start(out=wot_lo_sb[:], in_=wot_lo_d[:])
    if with_bias:
        b123_sb = const.tile([1, 3 * NHID], f32, name=f"b123{r}")
        nc.sync.dma_start(out=b123_sb[:], in_=b123_d[:])
        bo_sb = const.tile([1, NCLASS], f32, name=f"bo{r}")
        nc.sync.dma_start(out=bo_sb[:], in_=bo_d[:])
        ones_row = const.tile([1, 128], f32, name=f"ones{r}")
        nc.vector.memset(ones_row[:], 1.0)
    id128 = const.tile([128, 128], f32, name=f"id128_{r}")
    make_identity(nc, id128[:])

    # persistent stage-1 tensors
    h1a = [const.tile([128, nt, NHID], f32, name=f"h1a{k}_{r}") for k in range(3)]
    h1t = [const.tile([NHID, nloc], f32, name=f"h1t{k}_{r}") for k in range(3)]
    gcat = const.tile([NHID, 3 * NHID], f32, name=f"gcat{r}")
    mcat = const.tile([NHID, 3 * NHID], f32, name=f"mcat{r}")
    # hc = [elu(z1); elu(z2); elu(z3)] transposed, rows = 192 channels
    hc_hi = const.tile([128, nloc], f32, name=f"hc_hi{r}")
    hc_lo = const.tile([64, nloc], f32, name=f"hc_lo{r}")

    # ---- stage 1a: h = x.T @ W.T + b, row norms, h1 (both layouts), G ----
    g_ps = [ps_tile([NHID, NHID], f"acc{k}", f"g_ps{k}") for k in range(3)]
    for t in range(nt):
        ha_ps = ps_tile([128, 3 * NHID], "mm128", f"ha{t}", bufs=2)
        nc.tensor.matmul(
            ha_ps[:],
            x_sb[:, ts(t, 128)],
            w123t_sb[:],
            start=True,
            stop=not with_bias,
        )
        if with_bias:
            nc.tensor.matmul(ha_ps[:], ones_row[:], b123_sb[:], start=False, stop=True)
        for k in range(3):
            seg = ha_ps[:, ts(k, NHID)]
            scr = work.tile([128, NHID], f32, tag="scr1", name=f"scr1_{t}_{k}_{r}")
            q = small.tile([128, 1], f32, tag="q1", name=f"q1_{t}_{k}_{r}")
            nc.scalar.activation(
                scr[:], seg, mybir.ActivationFunctionType.Square, accum_out=q[:]
            )
            rn = _norm_scalars(nc, small, q, "h1")
            nc.vector.tensor_scalar_mul(h1a[k][:, t, :], seg, rn[:])
            # transpose into (d, n) layout
            tr_ps = ps_tile([NHID, 128], "tr", f"tr_{t}_{k}")
            nc.tensor.transpose(tr_ps[:], h1a[k][:, t, :], id128[:])
            nc.vector.tensor_copy(h1t[k][:, ts(t, 128)], tr_ps[:])
            # G_k partial accumulation (per-head range of one PSUM tile)
            nc.tensor.matmul(
                g_ps[k][:],
                h1a[k][:, t, :],
                h1a[k][:, t, :],
                start=(t == 0),
                stop=(t == nt - 1),
            )
    for k in range(3):
        nc.vector.tensor_copy(gcat[:, ts(k, NHID)], g_ps[k][:])

    # ---- global G ----
    if use_collectives:
        g_out = _allreduce(nc, dram, gcat, [NHID, 3 * NHID], f"g_{r}")
        g_glob = const.tile([NHID, 3 * NHID], f32, name=f"g_glob{r}")
        nc.sync.dma_start(out=g_glob[:], in_=g_out[:])
    else:
        g_glob = gcat

    # ---- stage 1b: column norms of e, M ----
    m_ps = [ps_tile([NHID, NHID], f"acc{k}", f"m_ps{k}") for k in range(3)]
    for t in range(nt):
        for k in range(3):
            ta_ps = ps_tile([128, NHID], "mm128", f"ta_{t}_{k}", bufs=2)
            nc.tensor.matmul(
                ta_ps[:],
                h1t[k][:, ts(t, 128)],
                g_glob[:, ts(k, NHID)],
                start=True,
                stop=True,
            )
            scr = work.tile([128, NHID], f32, tag="scr2", name=f"scr2_{t}_{k}_{r}")
            p = small.tile([128, 1], f32, tag="p1", name=f"p1_{t}_{k}_{r}")
            nc.vector.tensor_mul(scr[:], ta_ps[:], h1a[k][:, t, :])
            nc.vector.reduce_sum(p[:], scr[:], axis=mybir.AxisListType.X)
            icn = _norm_scalars(nc, small, p, "cn")
            h1s = work.tile([128, NHID], f32, tag="h1s", name=f"h1s_{t}_{k}_{r}")
            nc.vector.tensor_scalar_mul(h1s[:], h1a[k][:, t, :], icn[:])
            nc.tensor.matmul(
                m_ps[k][:],
                h1a[k][:, t, :],
                h1s[:],
                start=(t == 0),
                stop=(t == nt - 1),
            )
    for k in range(3):
        nc.vector.tensor_copy(mcat[:, ts(k, NHID)], m_ps[k][:])

    # ---- global M ----
    if use_collectives:
        m_out = _allreduce(nc, dram, mcat, [NHID, 3 * NHID], f"m_{r}")
        m_glob = const.tile([NHID, 3 * NHID], f32, name=f"m_glob{r}")
        nc.sync.dma_start(out=m_glob[:], in_=m_out[:])
    else:
        m_glob = mcat

    # ---- z = h1 @ M (transposed layout), elu -> hc ----
    for k in range(3):
        for c in range(nch):
            zt_ps = ps_tile([NHID, 512], "wide", f"zt_{k}_{c}")
            nc.tensor.matmul(
                zt_ps[:],
                m_glob[:, ts(k, NHID)],
                h1t[k][:, ts(c, 512)],
                start=True,
                stop=True,
            )
            # elu(z) = exp(min(z,0)) + (max(z,0) - 1)
            e_min = work.tile([NHID, 512], f32, tag="emin", name=f"emin_{k}_{c}_{r}")
            nc.vector.tensor_scalar_min(e_min[:], zt_ps[:], 0.0)
            e_exp = work.tile([NHID, 512], f32, tag="eexp", name=f"eexp_{k}_{c}_{r}")
            nc.scalar.activation(e_exp[:], e_min[:], mybir.ActivationFunctionType.Exp)
            e_max = work.tile([NHID, 512], f32, tag="emax", name=f"emax_{k}_{c}_{r}")
            nc.vector.tensor_scalar(
                out=e_max[:],
                in0=zt_ps[:],
                scalar1=0.0,
                scalar2=-1.0,
                op0=mybir.AluOpType.max,
                op1=mybir.AluOpType.add,
            )
            dst = (
                hc_hi[ts(k, NHID), ts(c, 512)]
                if k < 2
                else hc_lo[:, ts(c, 512)]
            )
            nc.vector.tensor_add(dst, e_exp[:], e_max[:])

    # ---- stage 2: output head (d = 16) ----
    h1oa = const.tile([128, nt, NCLASS], f32, name=f"h1oa{r}")
    h1ot = const.tile([NCLASS, nloc], f32, name=f"h1ot{r}")
    go_ps = ps_tile([NCLASS, NCLASS], "oacc", "go_ps")
    for t in range(nt):
        hoa_ps = ps_tile([128, NCLASS], "mm128", f"hoa_{t}", bufs=2)
        nc.tensor.matmul(
            hoa_ps[:], hc_hi[:, ts(t, 128)], wot_hi_sb[:], start=True, stop=False
        )
        nc.tensor.matmul(
            hoa_ps[:],
            hc_lo[:, ts(t, 128)],
            wot_lo_sb[:],
            start=False,
            stop=not with_bias,
        )
        if with_bias:
            nc.tensor.matmul(hoa_ps[:], ones_row[:], bo_sb[:], start=False, stop=True)
        scr = work.tile([128, NCLASS], f32, tag="scro", name=f"scro_{t}_{r}")
        q = small.tile([128, 1], f32, tag="qo", name=f"qo_{t}_{r}")
        nc.scalar.activation(
            scr[:], hoa_ps[:], mybir.ActivationFunctionType.Square, accum_out=q[:]
        )
        rno = _norm_scalars(nc, small, q, "ho")
        nc.vector.tensor_scalar_mul(h1oa[:, t, :], hoa_ps[:], rno[:])
        tro_ps = ps_tile([NCLASS, 128], "tr", f"tro_{t}")
        nc.tensor.transpose(tro_ps[:], h1oa[:, t, :], id128[:])
        nc.vector.tensor_copy(h1ot[:, ts(t, 128)], tro_ps[:])
        nc.tensor.matmul(
            go_ps[:],
            h1oa[:, t, :],
            h1oa[:, t, :],
            start=(t == 0),
            stop=(t == nt - 1),
        )
    go_sb = const.tile([NCLASS, NCLASS], f32, name=f"go_sb{r}")
    nc.vector.tensor_copy(go_sb[:], go_ps[:])

    if use_collectives:
        go_out = _allreduce(nc, dram, go_sb, [NCLASS, NCLASS], f"go_{r}")
        go_glob = const.tile([NCLASS, NCLASS], f32, name=f"go_glob{r}")
        nc.sync.dma_start(out=go_glob[:], in_=go_out[:])
    else:
        go_glob = go_sb

    mo_ps = ps_tile([NCLASS, NCLASS], "oacc", "mo_ps")
    for t in range(nt):
        to_ps = ps_tile([128, NCLASS], "mm128", f"to_{t}", bufs=2)
        nc.tensor.matmul(
            to_ps[:], h1ot[:, ts(t, 128)], go_glob[:], start=True, stop=True
        )
        scr = work.tile([128, NCLASS], f32, tag="scro2", name=f"scro2_{t}_{r}")
        p = small.tile([128, 1], f32, tag="po", name=f"po_{t}_{r}")
        nc.vector.tensor_mul(scr[:], to_ps[:], h1oa[:, t, :])
        nc.vector.reduce_sum(p[:], scr[:], axis=mybir.AxisListType.X)
        icno = _norm_scalars(nc, small, p, "cno")
        h1so = work.tile([128, NCLASS], f32, tag="h1so", name=f"h1so_{t}_{r}")
        nc.vector.tensor_scalar_mul(h1so[:], h1oa[:, t, :], icno[:])
        nc.tensor.matmul(
            mo_ps[:],
            h1oa[:, t, :],
            h1so[:],
            start=(t == 0),
            stop=(t == nt - 1),
        )
    mo_sb = const.tile([NCLASS, NCLASS], f32, name=f"mo_sb{r}")
    nc.vector.tensor_copy(mo_sb[:], mo_ps[:])

    if use_collectives:
        mo_out = _allreduce(nc, dram, mo_sb, [NCLASS, NCLASS], f"mo_{r}")
        mo_glob = const.tile([NCLASS, NCLASS], f32, name=f"mo_glob{r}")
        nc.sync.dma_start(out=mo_glob[:], in_=mo_out[:])
    else:
        mo_glob = mo_sb

    # ---- final: out = (h1o @ Mo).T = Mo.T @ h1o.T, no activation ----
    fot_sb = const.tile([NCLASS, nloc], f32, name=f"fot_sb{r}")
    for c in range(nch):
        fot_ps = ps_tile([NCLASS, 512], "wide", f"fot_{c}")
        nc.tensor.matmul(
            fot_ps[:], mo_glob[:], h1ot[:, ts(c, 512)], start=True, stop=True
        )
        nc.vector.tensor_copy(fot_sb[:, ts(c, 512)], fot_ps[:])
    nc.sync.dma_start(out=out_d[:], in_=fot_sb[:])


def build_program(reps=1, mode="rep", with_bias=False, loop=1):
    """Build the Bass program (shared by kernel() and test timing).

    loop > 1 wraps the body in an on-device For_i (timing amplification;
    only valid without collectives, i.e. mode="rep")."""
    key = (reps, mode, with_bias, loop)
    if key in _prog_cache:
        return _prog_cache[key]
    assert loop == 1 or mode == "rep", "device loop requires no collectives"

    _patch_tile_drain()
    import concourse.bass as bass
    import concourse.tile as tile
    import concourse.mybir as mybir
    from contextlib import ExitStack

    nloc = NLOC if mode == "shard" else N
    use_collectives = mode == "shard"

    f32 = mybir.dt.float32
    nc = bass.Bass(num_devices=N_CORES)
    tensors = {
        "xloc": nc.dram_tensor("xloc", [128, nloc], f32, kind="ExternalInput"),
        "w123t": nc.dram_tensor("w123t", [128, 3 * NHID], f32, kind="ExternalInput"),
        "b123": nc.dram_tensor("b123", [1, 3 * NHID], f32, kind="ExternalInput"),
        "wot_hi": nc.dram_tensor("wot_hi", [128, NCLASS], f32, kind="ExternalInput"),
        "wot_lo": nc.dram_tensor("wot_lo", [64, NCLASS], f32, kind="ExternalInput"),
        "bo": nc.dram_tensor("bo", [1, NCLASS], f32, kind="ExternalInput"),
        "outt": nc.dram_tensor("outt", [NCLASS, nloc], f32, kind="ExternalOutput"),
    }

    with tile.TileContext(nc) as tc:
        if loop > 1:
            with tc.For_i(0, loop, 1):
                for r in range(reps):
                    with ExitStack() as ctx:
                        _emit_body(
                            nc, tc, ctx, tensors, r, nloc, use_collectives, with_bias
                        )
        else:
            for r in range(reps):
                with ExitStack() as ctx:
                    _emit_body(
                        nc, tc, ctx, tensors, r, nloc, use_collectives, with_bias
                    )

    _split_multi_waits(nc)
    _prog_cache[key] = nc
    return nc


def make_in_maps(x, W1, b1, W2, b2, W3, b3, Wo, bo, mode="rep"):
    x_mem = np.asarray(x, dtype=np.float32).reshape(NFEAT, N)
    w123t = np.ascontiguousarray(
        np.concatenate(
            [np.asarray(W1).T, np.asarray(W2).T, np.asarray(W3).T], axis=1
        ),
        dtype=np.float32,
    )
    b123 = (
        np.concatenate([np.asarray(b1), np.asarray(b2), np.asarray(b3)])
        .reshape(1, 3 * NHID)
        .astype(np.float32)
    )
    wot = np.ascontiguousarray(np.asarray(Wo).T, dtype=np.float32)  # (192, 16)
    wot_hi = np.ascontiguousarray(wot[:128])
    wot_lo = np.ascontiguousarray(wot[128:])
    bo_r = np.asarray(bo).reshape(1, NCLASS).astype(np.float32)
    common = {
        "w123t": w123t,
        "b123": b123,
        "wot_hi": wot_hi,
        "wot_lo": wot_lo,
        "bo": bo_r,
    }
    in_maps = []
    for c in range(N_CORES):
        if mode == "shard":
            xc = np.ascontiguousarray(x_mem[:, c * NLOC : (c + 1) * NLOC])
        else:
            xc = x_mem
        in_maps.append({"xloc": xc, **common})
    return in_maps


def assemble_output(results, mode="rep"):
    if mode == "shard":
        slabs = [results[c]["outt"] for c in range(N_CORES)]
        full = np.concatenate(slabs, axis=1)  # (16, 4096)
    else:
        full = results[0]["outt"]
    return np.ascontiguousarray(full.reshape(1, NCLASS, 64, 64), dtype=np.float32)


def kernel(x, W1, b1, W2, b2, W3, b3, Wo, bo):
    from concourse.bass_utils import run_bass_kernel_spmd

    mode = "rep"
    with_bias = any(
        np.any(np.asarray(b)) for b in (b1, b2, b3, bo)
    )
    nc = build_program(reps=1, mode=mode, with_bias=with_bias)
    in_maps = make_in_maps(x, W1, b1, W2, b2, W3, b3, Wo, bo, mode=mode)
    res = run_bass_kernel_spmd(nc, in_maps, list(range(N_CORES)))
    return assemble_output(res.results, mode=mode)


# revision 16
# speedup vs baseline: 1.1278x; 1.1278x over previous
"""GAT-style kernel for Trainium2, 8 NeuronCores.

Math (per head, d = nhid):
    h  = xf @ W.T + b                  (N, d)
    h1 = h / max(||h||_row, eps)       row L2 normalize
    e  = h1 @ h1.T                     (N, N)  -- never materialized
    att = e / ||e||_col                column L2 normalize
    out = act(att @ h1)

Collapse: with G = h1.T @ h1 (d x d),
    ||e||_col[j]^2 = h1_j.T G h1_j     (row-wise quadratic form)
    att @ h1 = h1 @ M,  M = h1.T @ (h1 / cn)   (d x d)
so the N x N attention matrix is never formed and the whole computation
is O(N d^2).

Two modes:
  * "rep"  (default): every core gets the full input and computes the
    full output; no collectives.  On this stack AllReduce costs ~700us
    per call, far more than the whole collapsed computation, so
    replication wins despite 8x redundant compute.
  * "shard": row-shard N across the 8 cores, 4 small AllReduces
    (G, M for the hidden heads batched; G_o, M_o for the output head).
"""

import sys

for _p in ("/opt/trn_rl_repo", "/root/.axon_site/_ro/trn_rl_repo"):
    if _p not in sys.path:
        sys.path.append(_p)

import numpy as np

N_CORES = 8
N = 4096
NLOC = N // N_CORES          # 512 rows per core in shard mode
NFEAT = 128
NHID = 64
NCLASS = 16
EPS = 1e-12

_prog_cache = {}


def _patch_tile_drain():
    """Walrus in this container rejects Tile's tail drain (too many sync
    waits on one instruction).  Split it into one-wait-per-drain."""
    import concourse.tile as tile
    from concourse.vector_clock import ScopedClock, VectorClock

    if getattr(tile.TileContext, "_drain_split_patched", False):
        return

    def _drain_and_barrier(self, tick_clock, wait_clock):
        nc = self.nc
        gvc = tick_clock.global_clock  # VectorClock
        n = len(gvc)
        for proc in range(n):
            t = gvc[proc]
            if t > 0:
                sub = VectorClock([t if i == proc else 0 for i in range(n)])
                d = nc.sync.drain()
                wait_clock.add_sem_waits(d.ins, ScopedClock({None: sub}))
        nc.all_engine_barrier()
        assert self.sems is not None
        popped = nc._tile_sem_poison_stack.pop()
        assert popped is self._sem_poison
        nc.clear_and_free_semaphores(list(self.sems.allocated().values()))
        nc.all_engine_barrier()

    tile.TileContext._drain_and_barrier = _drain_and_barrier
    tile.TileContext._drain_split_patched = True


def _split_multi_waits(nc):
    """This container's walrus allows only one sync-wait per instruction.
    Hoist extra waits onto standalone same-engine NoOps."""
    import concourse.mybir as mybir

    n_new = 0
    for blk in nc.main_func.blocks:
        out = []
        changed = False
        for inst in blk.instructions:
            si = inst.sync_info
            waits = list(si.on_wait) if (si and si.on_wait) else []
            if len(waits) > 1:
                changed = True
                for w in waits[:-1]:
                    nop = mybir.InstNoOp(name=f"{inst.name}-xw{n_new}", ins=[], outs=[])
                    n_new += 1
                    nop.engine = inst.engine
                    nop.sync_info = mybir.SyncInfo(on_wait=[w], on_update=[])
                    out.append(nop)
                si.on_wait = [waits[-1]]
                inst.sync_info = si
            out.append(inst)
        if changed:
            blk.instructions = out


def _norm_scalars(nc, pool, q, name_tag, width=1):
    """q (128,w) sums of squares -> rn (128,w) = 1/max(sqrt(q), EPS).

    Computed as sqrt(1/max(q, EPS^2)), identical in exact arithmetic and
    grouping the DVE ops together (max, recip on DVE; sqrt on ACT)."""
    import concourse.mybir as mybir

    f32 = mybir.dt.float32
    qm = pool.tile([128, width], f32, tag=f"qm_{name_tag}", name=f"qm_{name_tag}")
    nc.vector.tensor_scalar_max(qm[:], q[:], EPS * EPS)
    qi = pool.tile([128, width], f32, tag=f"qi_{name_tag}", name=f"qi_{name_tag}")
    nc.vector.reciprocal(qi[:], qm[:])
    rn = pool.tile([128, width], f32, tag=f"rn_{name_tag}", name=f"rn_{name_tag}")
    nc.scalar.sqrt(rn[:], qi[:])
    return rn


def _allreduce(nc, dram_pool, sbuf_in, shape, tag):
    """AllReduce sbuf_in (shape) across all cores via DRAM bounce buffers."""
    import concourse.mybir as mybir

    bnc_in = dram_pool.tile(shape, mybir.dt.float32, name=f"arin_{tag}")
    bnc_out = dram_pool.tile(
        shape, mybir.dt.float32, addr_space="Shared", name=f"arout_{tag}"
    )
    nc.sync.dma_start(out=bnc_in[:], in_=sbuf_in[:])
    nc.gpsimd.collective_compute(
        "AllReduce",
        mybir.AluOpType.add,
        replica_groups=[list(range(N_CORES))],
        ins=[bnc_in.opt()],
        outs=[bnc_out.opt()],
    )
    return bnc_out


def _emit_body(nc, tc, ctx, tensors, rep, nloc, use_collectives, with_bias):
    import concourse.mybir as mybir
    from concourse.bass import ts
    from concourse.masks import make_identity

    f32 = mybir.dt.float32
    r = rep
    nt = nloc // 128        # row tiles
    nch = nloc // 512       # 512-wide column chunks in transposed layout

    x_d = tensors["xloc"]
    w123t_d = tensors["w123t"]
    b123_d = tensors["b123"]
    wot_hi_d = tensors["wot_hi"]
    wot_lo_d = tensors["wot_lo"]
    bo_d = tensors["bo"]
    out_d = tensors["outt"]

    const = ctx.enter_context(tc.tile_pool(name=f"const{r}", bufs=1))
    work = ctx.enter_context(tc.tile_pool(name=f"work{r}", bufs=2))
    small = ctx.enter_context(tc.tile_pool(name=f"small{r}", bufs=2))
    # PSUM budget is 8 banks; tags below sum to exactly 8:
    #   mm128 (2) + tr (1) + wide (1) + acc0/1/2 (3) + oacc (1)
    psum = ctx.enter_context(tc.tile_pool(name=f"psum{r}", bufs=1, space="PSUM"))
    dram = ctx.enter_context(tc.tile_pool(name=f"dram{r}", bufs=1, space="DRAM"))

    def ps_tile(shape, tag, name, bufs=1):
        return psum.tile(
            shape, mybir.dt.float32, tag=tag, name=f"{name}_{r}", bufs=bufs
        )

    # ---- constants / inputs to SBUF ----
    x_sb = const.tile([128, nloc], f32, name=f"x_sb{r}")
    nc.sync.dma_start(out=x_sb[:], in_=x_d[:])
    w123t_sb = const.tile([128, 3 * NHID], f32, name=f"w123t{r}")
    nc.sync.dma_start(out=w123t_sb[:], in_=w123t_d[:])
    wot_hi_sb = const.tile([128, NCLASS], f32, name=f"wot_hi{r}")
    nc.sync.dma_start(out=wot_hi_sb[:], in_=wot_hi_d[:])
    wot_lo_sb = const.tile([64, NCLASS], f32, name=f"wot_lo{r}")
    nc.sync.dma_start(out=wot_lo_sb[:], in_=wot_lo_d[:])
    if with_bias:
        b123_sb = const.tile([1, 3 * NHID], f32, name=f"b123{r}")
        nc.sync.dma_start(out=b123_sb[:], in_=b123_d[:])
        bo_sb = const.tile([1, NCLASS], f32, name=f"bo{r}")
        nc.sync.dma_start(out=bo_sb[:], in_=bo_d[:])
        ones_row = const.tile([1, 128], f32, name=f"ones{r}")
        nc.vector.memset(ones_row[:], 1.0)
    id128 = const.tile([128, 128], f32, name=f"id128_{r}")
    make_identity(nc, id128[:])

    # persistent stage-1 tensors
    h1a = const.tile([128, nt, 3 * NHID], f32, name=f"h1a_{r}")
    h1t = [const.tile([NHID, nloc], f32, name=f"h1t{k}_{r}") for k in range(3)]
    gcat = const.tile([NHID, 3 * NHID], f32, name=f"gcat{r}")
    mcat = const.tile([NHID, 3 * NHID], f32, name=f"mcat{r}")
    # hc = [elu(z1); elu(z2); elu(z3)] transposed, rows = 192 channels
    hc_hi = const.tile([128, nloc], f32, name=f"hc_hi{r}")
    hc_lo = const.tile([64, nloc], f32, name=f"hc_lo{r}")

    # ---- stage 1a: h = x.T @ W.T + b, row norms, h1 (both layouts), G ----
    g_ps = [ps_tile([NHID, NHID], f"acc{k}", f"g_ps{k}") for k in range(3)]
    for t in range(nt):
        ha_ps = ps_tile([128, 3 * NHID], "mm128", f"ha{t}", bufs=2)
        nc.tensor.matmul(
            ha_ps[:],
            x_sb[:, ts(t, 128)],
            w123t_sb[:],
            start=True,
            stop=not with_bias,
        )
        if with_bias:
            nc.tensor.matmul(ha_ps[:], ones_row[:], b123_sb[:], start=False, stop=True)
        scr = work.tile([128, 3 * NHID], f32, tag="scr1", name=f"scr1_{t}_{r}")
        nc.scalar.activation(scr[:], ha_ps[:], mybir.ActivationFunctionType.Square)
        q3 = small.tile([128, 3], f32, tag="q3", name=f"q3_{t}_{r}")
        nc.vector.reduce_sum(
            q3[:],
            scr[:].rearrange("p (k d) -> p k d", k=3),
            axis=mybir.AxisListType.X,
        )
        rn3 = _norm_scalars(nc, small, q3, "h1", width=3)
        for k in range(3):
            nc.vector.tensor_scalar_mul(
                h1a[:, t, ts(k, NHID)], ha_ps[:, ts(k, NHID)], rn3[:, k : k + 1]
            )
        for k in range(3):
            # transpose into (d, n) layout
            tr_ps = ps_tile([NHID, 128], "tr", f"tr_{t}_{k}")
            nc.tensor.transpose(tr_ps[:], h1a[:, t, ts(k, NHID)], id128[:])
            nc.vector.tensor_copy(h1t[k][:, ts(t, 128)], tr_ps[:])
            # G_k partial accumulation
            nc.tensor.matmul(
                g_ps[k][:],
                h1a[:, t, ts(k, NHID)],
                h1a[:, t, ts(k, NHID)],
                start=(t == 0),
                stop=(t == nt - 1),
            )
    for k in range(3):
        nc.vector.tensor_copy(gcat[:, ts(k, NHID)], g_ps[k][:])

    # ---- global G ----
    if use_collectives:
        g_out = _allreduce(nc, dram, gcat, [NHID, 3 * NHID], f"g_{r}")
        g_glob = const.tile([NHID, 3 * NHID], f32, name=f"g_glob{r}")
        nc.sync.dma_start(out=g_glob[:], in_=g_out[:])
    else:
        g_glob = gcat

    # ---- stage 1b: column norms of e, M ----
    m_ps = [ps_tile([NHID, NHID], f"acc{k}", f"m_ps{k}") for k in range(3)]
    for t in range(nt):
        ta_ps = ps_tile([128, 3 * NHID], "mm128", f"ta_{t}", bufs=2)
        for k in range(3):
            nc.tensor.matmul(
                ta_ps[:, ts(k, NHID)],
                h1t[k][:, ts(t, 128)],
                g_glob[:, ts(k, NHID)],
                start=True,
                stop=True,
            )
        scr = work.tile([128, 3 * NHID], f32, tag="scr2", name=f"scr2_{t}_{r}")
        nc.vector.tensor_mul(scr[:], ta_ps[:], h1a[:, t, :])
        p3 = small.tile([128, 3], f32, tag="p3", name=f"p3_{t}_{r}")
        nc.vector.reduce_sum(
            p3[:],
            scr[:].rearrange("p (k d) -> p k d", k=3),
            axis=mybir.AxisListType.X,
        )
        icn3 = _norm_scalars(nc, small, p3, "cn", width=3)
        h1s = work.tile([128, 3 * NHID], f32, tag="h1s", name=f"h1s_{t}_{r}")
        for k in range(3):
            nc.vector.tensor_scalar_mul(
                h1s[:, ts(k, NHID)], h1a[:, t, ts(k, NHID)], icn3[:, k : k + 1]
            )
        for k in range(3):
            nc.tensor.matmul(
                m_ps[k][:],
                h1a[:, t, ts(k, NHID)],
                h1s[:, ts(k, NHID)],
                start=(t == 0),
                stop=(t == nt - 1),
            )
    for k in range(3):
        nc.vector.tensor_copy(mcat[:, ts(k, NHID)], m_ps[k][:])

    # ---- global M ----
    if use_collectives:
        m_out = _allreduce(nc, dram, mcat, [NHID, 3 * NHID], f"m_{r}")
        m_glob = const.tile([NHID, 3 * NHID], f32, name=f"m_glob{r}")
        nc.sync.dma_start(out=m_glob[:], in_=m_out[:])
    else:
        m_glob = mcat

    # ---- z = h1 @ M (transposed layout), elu -> hc ----
    for k in range(3):
        for c in range(nch):
            zt_ps = ps_tile([NHID, 512], "wide", f"zt_{k}_{c}")
            nc.tensor.matmul(
                zt_ps[:],
                m_glob[:, ts(k, NHID)],
                h1t[k][:, ts(c, 512)],
                start=True,
                stop=True,
            )
            # elu(z) = exp(min(z,0)) + (max(z,0) - 1)
            e_min = work.tile([NHID, 512], f32, tag="emin", name=f"emin_{k}_{c}_{r}")
            nc.vector.tensor_scalar_min(e_min[:], zt_ps[:], 0.0)
            e_exp = work.tile([NHID, 512], f32, tag="eexp", name=f"eexp_{k}_{c}_{r}")
            nc.scalar.activation(e_exp[:], e_min[:], mybir.ActivationFunctionType.Exp)
            e_max = work.tile([NHID, 512], f32, tag="emax", name=f"emax_{k}_{c}_{r}")
            nc.vector.tensor_scalar(
                out=e_max[:],
                in0=zt_ps[:],
                scalar1=0.0,
                scalar2=-1.0,
                op0=mybir.AluOpType.max,
                op1=mybir.AluOpType.add,
            )
            dst = (
                hc_hi[ts(k, NHID), ts(c, 512)]
                if k < 2
                else hc_lo[:, ts(c, 512)]
            )
            nc.vector.tensor_add(dst, e_exp[:], e_max[:])

    # ---- stage 2: output head (d = 16) ----
    h1oa = const.tile([128, nt, NCLASS], f32, name=f"h1oa{r}")
    h1ot = const.tile([NCLASS, nloc], f32, name=f"h1ot{r}")
    go_ps = ps_tile([NCLASS, NCLASS], "oacc", "go_ps")
    for t in range(nt):
        hoa_ps = ps_tile([128, NCLASS], "mm128", f"hoa_{t}", bufs=2)
        nc.tensor.matmul(
            hoa_ps[:], hc_hi[:, ts(t, 128)], wot_hi_sb[:], start=True, stop=False
        )
        nc.tensor.matmul(
            hoa_ps[:],
            hc_lo[:, ts(t, 128)],
            wot_lo_sb[:],
            start=False,
            stop=not with_bias,
        )
        if with_bias:
            nc.tensor.matmul(hoa_ps[:], ones_row[:], bo_sb[:], start=False, stop=True)
        scr = work.tile([128, NCLASS], f32, tag="scro", name=f"scro_{t}_{r}")
        q = small.tile([128, 1], f32, tag="qo", name=f"qo_{t}_{r}")
        nc.scalar.activation(
            scr[:], hoa_ps[:], mybir.ActivationFunctionType.Square, accum_out=q[:]
        )
        rno = _norm_scalars(nc, small, q, "ho")
        nc.vector.tensor_scalar_mul(h1oa[:, t, :], hoa_ps[:], rno[:])
        tro_ps = ps_tile([NCLASS, 128], "tr", f"tro_{t}")
        nc.tensor.transpose(tro_ps[:], h1oa[:, t, :], id128[:])
        nc.vector.tensor_copy(h1ot[:, ts(t, 128)], tro_ps[:])
        nc.tensor.matmul(
            go_ps[:],
            h1oa[:, t, :],
            h1oa[:, t, :],
            start=(t == 0),
            stop=(t == nt - 1),
        )
    go_sb = const.tile([NCLASS, NCLASS], f32, name=f"go_sb{r}")
    nc.vector.tensor_copy(go_sb[:], go_ps[:])

    if use_collectives:
        go_out = _allreduce(nc, dram, go_sb, [NCLASS, NCLASS], f"go_{r}")
        go_glob = const.tile([NCLASS, NCLASS], f32, name=f"go_glob{r}")
        nc.sync.dma_start(out=go_glob[:], in_=go_out[:])
    else:
        go_glob = go_sb

    mo_ps = ps_tile([NCLASS, NCLASS], "oacc", "mo_ps")
    for t in range(nt):
        to_ps = ps_tile([128, NCLASS], "mm128", f"to_{t}", bufs=2)
        nc.tensor.matmul(
            to_ps[:], h1ot[:, ts(t, 128)], go_glob[:], start=True, stop=True
        )
        scr = work.tile([128, NCLASS], f32, tag="scro2", name=f"scro2_{t}_{r}")
        p = small.tile([128, 1], f32, tag="po", name=f"po_{t}_{r}")
        nc.vector.tensor_mul(scr[:], to_ps[:], h1oa[:, t, :])
        nc.vector.reduce_sum(p[:], scr[:], axis=mybir.AxisListType.X)
        icno = _norm_scalars(nc, small, p, "cno")
        h1so = work.tile([128, NCLASS], f32, tag="h1so", name=f"h1so_{t}_{r}")
        nc.vector.tensor_scalar_mul(h1so[:], h1oa[:, t, :], icno[:])
        nc.tensor.matmul(
            mo_ps[:],
            h1oa[:, t, :],
            h1so[:],
            start=(t == 0),
            stop=(t == nt - 1),
        )
    mo_sb = const.tile([NCLASS, NCLASS], f32, name=f"mo_sb{r}")
    nc.vector.tensor_copy(mo_sb[:], mo_ps[:])

    if use_collectives:
        mo_out = _allreduce(nc, dram, mo_sb, [NCLASS, NCLASS], f"mo_{r}")
        mo_glob = const.tile([NCLASS, NCLASS], f32, name=f"mo_glob{r}")
        nc.sync.dma_start(out=mo_glob[:], in_=mo_out[:])
    else:
        mo_glob = mo_sb

    # ---- final: out = (h1o @ Mo).T = Mo.T @ h1o.T, no activation ----
    fot_sb = const.tile([NCLASS, nloc], f32, name=f"fot_sb{r}")
    for c in range(nch):
        fot_ps = ps_tile([NCLASS, 512], "wide", f"fot_{c}")
        nc.tensor.matmul(
            fot_ps[:], mo_glob[:], h1ot[:, ts(c, 512)], start=True, stop=True
        )
        nc.vector.tensor_copy(fot_sb[:, ts(c, 512)], fot_ps[:])
    nc.sync.dma_start(out=out_d[:], in_=fot_sb[:])


def build_program(reps=1, mode="rep", with_bias=False, loop=1):
    """Build the Bass program (shared by kernel() and test timing).

    loop > 1 wraps the body in an on-device For_i (timing amplification;
    only valid without collectives, i.e. mode="rep")."""
    key = (reps, mode, with_bias, loop)
    if key in _prog_cache:
        return _prog_cache[key]
    assert loop == 1 or mode == "rep", "device loop requires no collectives"

    _patch_tile_drain()
    import concourse.bass as bass
    import concourse.tile as tile
    import concourse.mybir as mybir
    from contextlib import ExitStack

    nloc = NLOC if mode == "shard" else N
    use_collectives = mode == "shard"

    f32 = mybir.dt.float32
    nc = bass.Bass(num_devices=N_CORES)
    tensors = {
        "xloc": nc.dram_tensor("xloc", [128, nloc], f32, kind="ExternalInput"),
        "w123t": nc.dram_tensor("w123t", [128, 3 * NHID], f32, kind="ExternalInput"),
        "b123": nc.dram_tensor("b123", [1, 3 * NHID], f32, kind="ExternalInput"),
        "wot_hi": nc.dram_tensor("wot_hi", [128, NCLASS], f32, kind="ExternalInput"),
        "wot_lo": nc.dram_tensor("wot_lo", [64, NCLASS], f32, kind="ExternalInput"),
        "bo": nc.dram_tensor("bo", [1, NCLASS], f32, kind="ExternalInput"),
        "outt": nc.dram_tensor("outt", [NCLASS, nloc], f32, kind="ExternalOutput"),
    }

    with tile.TileContext(nc) as tc:
        if loop > 1:
            with tc.For_i(0, loop, 1):
                for r in range(reps):
                    with ExitStack() as ctx:
                        _emit_body(
                            nc, tc, ctx, tensors, r, nloc, use_collectives, with_bias
                        )
        else:
            for r in range(reps):
                with ExitStack() as ctx:
                    _emit_body(
                        nc, tc, ctx, tensors, r, nloc, use_collectives, with_bias
                    )

    _split_multi_waits(nc)
    _prog_cache[key] = nc
    return nc


def make_in_maps(x, W1, b1, W2, b2, W3, b3, Wo, bo, mode="rep"):
    x_mem = np.asarray(x, dtype=np.float32).reshape(NFEAT, N)
    w123t = np.ascontiguousarray(
        np.concatenate(
            [np.asarray(W1).T, np.asarray(W2).T, np.asarray(W3).T], axis=1
        ),
        dtype=np.float32,
    )
    b123 = (
        np.concatenate([np.asarray(b1), np.asarray(b2), np.asarray(b3)])
        .reshape(1, 3 * NHID)
        .astype(np.float32)
    )
    wot = np.ascontiguousarray(np.asarray(Wo).T, dtype=np.float32)  # (192, 16)
    wot_hi = np.ascontiguousarray(wot[:128])
    wot_lo = np.ascontiguousarray(wot[128:])
    bo_r = np.asarray(bo).reshape(1, NCLASS).astype(np.float32)
    common = {
        "w123t": w123t,
        "b123": b123,
        "wot_hi": wot_hi,
        "wot_lo": wot_lo,
        "bo": bo_r,
    }
    in_maps = []
    for c in range(N_CORES):
        if mode == "shard":
            xc = np.ascontiguousarray(x_mem[:, c * NLOC : (c + 1) * NLOC])
        else:
            xc = x_mem
        in_maps.append({"xloc": xc, **common})
    return in_maps


def assemble_output(results, mode="rep"):
    if mode == "shard":
        slabs = [results[c]["outt"] for c in range(N_CORES)]
        full = np.concatenate(slabs, axis=1)  # (16, 4096)
    else:
        full = results[0]["outt"]
    return np.ascontiguousarray(full.reshape(1, NCLASS, 64, 64), dtype=np.float32)


def kernel(x, W1, b1, W2, b2, W3, b3, Wo, bo):
    from concourse.bass_utils import run_bass_kernel_spmd

    mode = "rep"
    with_bias = any(
        np.any(np.asarray(b)) for b in (b1, b2, b3, bo)
    )
    nc = build_program(reps=1, mode=mode, with_bias=with_bias)
    in_maps = make_in_maps(x, W1, b1, W2, b2, W3, b3, Wo, bo, mode=mode)
    res = run_bass_kernel_spmd(nc, in_maps, list(range(N_CORES)))
    return assemble_output(res.results, mode=mode)


# revision 23
# speedup vs baseline: 1.3499x; 1.1969x over previous
"""GAT-style kernel for Trainium2, 8 NeuronCores.

Math (per head, d = nhid):
    h  = xf @ W.T + b                  (N, d)
    h1 = h / max(||h||_row, eps)       row L2 normalize
    e  = h1 @ h1.T                     (N, N)  -- never materialized
    att = e / ||e||_col                column L2 normalize
    out = act(att @ h1)

Collapse: with G = h1.T @ h1 (d x d),
    ||e||_col[j]^2 = h1_j.T G h1_j     (row-wise quadratic form)
    att @ h1 = h1 @ M,  M = h1.T @ (h1 / cn)   (d x d)
so the N x N attention matrix is never formed and the whole computation
is O(N d^2).

Two modes:
  * "rep"  (default): every core gets the full input and computes the
    full output; no collectives.  On this stack AllReduce costs ~700us
    per call, far more than the whole collapsed computation, so
    replication wins despite 8x redundant compute.
  * "shard": row-shard N across the 8 cores, 4 small AllReduces
    (G, M for the hidden heads batched; G_o, M_o for the output head).
"""

import sys

for _p in ("/opt/trn_rl_repo", "/root/.axon_site/_ro/trn_rl_repo"):
    if _p not in sys.path:
        sys.path.append(_p)

import numpy as np

N_CORES = 8
N = 4096
NLOC = N // N_CORES          # 512 rows per core in shard mode
NFEAT = 128
NHID = 64
NCLASS = 16
EPS = 1e-12

_prog_cache = {}
PHASES = {1, 2, 3, 4, 5, 6}  # surgical-profiling switch (timing experiments)


def _patch_tile_drain():
    """Walrus in this container rejects Tile's tail drain (too many sync
    waits on one instruction).  Split it into one-wait-per-drain."""
    import concourse.tile as tile
    from concourse.vector_clock import ScopedClock, VectorClock

    if getattr(tile.TileContext, "_drain_split_patched", False):
        return

    def _drain_and_barrier(self, tick_clock, wait_clock):
        nc = self.nc
        gvc = tick_clock.global_clock  # VectorClock
        n = len(gvc)
        for proc in range(n):
            t = gvc[proc]
            if t > 0:
                sub = VectorClock([t if i == proc else 0 for i in range(n)])
                d = nc.sync.drain()
                wait_clock.add_sem_waits(d.ins, ScopedClock({None: sub}))
        nc.all_engine_barrier()
        assert self.sems is not None
        popped = nc._tile_sem_poison_stack.pop()
        assert popped is self._sem_poison
        nc.clear_and_free_semaphores(list(self.sems.allocated().values()))
        nc.all_engine_barrier()

    tile.TileContext._drain_and_barrier = _drain_and_barrier
    tile.TileContext._drain_split_patched = True


def _split_multi_waits(nc):
    """This container's walrus allows only one sync-wait per instruction.
    Hoist extra waits onto standalone same-engine NoOps."""
    import concourse.mybir as mybir

    n_new = 0
    for blk in nc.main_func.blocks:
        out = []
        changed = False
        for inst in blk.instructions:
            si = inst.sync_info
            waits = list(si.on_wait) if (si and si.on_wait) else []
            if len(waits) > 1:
                changed = True
                for w in waits[:-1]:
                    nop = mybir.InstNoOp(name=f"{inst.name}-xw{n_new}", ins=[], outs=[])
                    n_new += 1
                    nop.engine = inst.engine
                    nop.sync_info = mybir.SyncInfo(on_wait=[w], on_update=[])
                    out.append(nop)
                si.on_wait = [waits[-1]]
                inst.sync_info = si
            out.append(inst)
        if changed:
            blk.instructions = out


def _norm_scalars(nc, pool, q, name_tag, width=1):
    """q (128,w) sums of squares -> rn (128,w) = 1/max(sqrt(q), EPS).

    Computed as sqrt(1/max(q, EPS^2)), identical in exact arithmetic and
    grouping the DVE ops together (max, recip on DVE; sqrt on ACT)."""
    import concourse.mybir as mybir

    f32 = mybir.dt.float32
    qm = pool.tile([128, width], f32, tag=f"qm_{name_tag}", name=f"qm_{name_tag}")
    nc.vector.tensor_scalar_max(qm[:], q[:], EPS * EPS)
    qi = pool.tile([128, width], f32, tag=f"qi_{name_tag}", name=f"qi_{name_tag}")
    nc.vector.reciprocal(qi[:], qm[:])
    rn = pool.tile([128, width], f32, tag=f"rn_{name_tag}", name=f"rn_{name_tag}")
    nc.scalar.sqrt(rn[:], qi[:])
    return rn


def _allreduce(nc, dram_pool, sbuf_in, shape, tag):
    """AllReduce sbuf_in (shape) across all cores via DRAM bounce buffers."""
    import concourse.mybir as mybir

    bnc_in = dram_pool.tile(shape, mybir.dt.float32, name=f"arin_{tag}")
    bnc_out = dram_pool.tile(
        shape, mybir.dt.float32, addr_space="Shared", name=f"arout_{tag}"
    )
    nc.sync.dma_start(out=bnc_in[:], in_=sbuf_in[:])
    nc.gpsimd.collective_compute(
        "AllReduce",
        mybir.AluOpType.add,
        replica_groups=[list(range(N_CORES))],
        ins=[bnc_in.opt()],
        outs=[bnc_out.opt()],
    )
    return bnc_out


def _emit_body(nc, tc, ctx, tensors, rep, nloc, use_collectives, with_bias):
    import concourse.mybir as mybir
    from concourse.bass import ts
    from concourse.masks import make_identity

    f32 = mybir.dt.float32
    r = rep
    nt = nloc // 128        # row tiles
    nch = nloc // 512       # 512-wide column chunks in transposed layout

    x_d = tensors["xloc"]
    w123t_d = tensors["w123t"]
    b123_d = tensors["b123"]
    wot_hi_d = tensors["wot_hi"]
    wot_lo_d = tensors["wot_lo"]
    bo_d = tensors["bo"]
    out_d = tensors["outt"]

    const = ctx.enter_context(tc.tile_pool(name=f"const{r}", bufs=1))
    work = ctx.enter_context(tc.tile_pool(name=f"work{r}", bufs=2))
    small = ctx.enter_context(tc.tile_pool(name=f"small{r}", bufs=2))
    # PSUM budget is 8 banks; tags below sum to exactly 8:
    #   mm128 (2) + tr (1) + wide (1) + acc0/1/2 (3) + oacc (1)
    psum = ctx.enter_context(tc.tile_pool(name=f"psum{r}", bufs=1, space="PSUM"))
    dram = ctx.enter_context(tc.tile_pool(name=f"dram{r}", bufs=1, space="DRAM"))

    def ps_tile(shape, tag, name, bufs=1):
        return psum.tile(
            shape, mybir.dt.float32, tag=tag, name=f"{name}_{r}", bufs=bufs
        )

    # ---- constants / inputs to SBUF ----
    x_sb = const.tile([128, nloc], f32, name=f"x_sb{r}")
    nc.sync.dma_start(out=x_sb[:], in_=x_d[:])
    w123t_sb = const.tile([128, 3 * NHID], f32, name=f"w123t{r}")
    nc.sync.dma_start(out=w123t_sb[:], in_=w123t_d[:])
    wot_hi_sb = const.tile([128, NCLASS], f32, name=f"wot_hi{r}")
    nc.sync.dma_start(out=wot_hi_sb[:], in_=wot_hi_d[:])
    wot_lo_sb = const.tile([64, NCLASS], f32, name=f"wot_lo{r}")
    nc.sync.dma_start(out=wot_lo_sb[:], in_=wot_lo_d[:])
    if with_bias:
        b123_sb = const.tile([1, 3 * NHID], f32, name=f"b123{r}")
        nc.sync.dma_start(out=b123_sb[:], in_=b123_d[:])
        bo_sb = const.tile([1, NCLASS], f32, name=f"bo{r}")
        nc.sync.dma_start(out=bo_sb[:], in_=bo_d[:])
        ones_row = const.tile([1, 128], f32, name=f"ones{r}")
        nc.vector.memset(ones_row[:], 1.0)
    id128 = const.tile([128, 128], f32, name=f"id128_{r}")
    make_identity(nc, id128[:])

    # persistent stage-1 tensors; heads 0,1 ride together on 128 partitions
    h1a = const.tile([128, nt, 3 * NHID], f32, name=f"h1a_{r}")
    h1t01 = const.tile([128, nloc], f32, name=f"h1t01_{r}")
    h1t2 = const.tile([NHID, nloc], f32, name=f"h1t2_{r}")
    # hc = [elu(z1); elu(z2); elu(z3)] transposed, rows = 192 channels
    hc_hi = const.tile([128, nloc], f32, name=f"hc_hi{r}")
    hc_lo = const.tile([64, nloc], f32, name=f"hc_lo{r}")

    import concourse.bass as bass

    def bcast_free(ap, inner):
        """Broadcast an AP with a trailing stride-0 inner dim."""
        return bass.AP(tensor=ap.tensor, offset=ap.offset, ap=[*ap.ap, [0, inner]])

    # ---- stage 1a: h = x.T @ W.T + b, row norms, h1 (both layouts), G ----
    # Heads 0,1 are processed as one 128-partition block wherever possible;
    # head 2 rides separately on 64 partitions.
    g01_ps = ps_tile([128, 128], "acc01", "g01") if 1 in PHASES else None
    g2_ps = ps_tile([NHID, NHID], "acc2", "g2") if 1 in PHASES else None
    for t in range(nt) if 1 in PHASES else []:
        ha_ps = ps_tile([128, 3 * NHID], "mm128", f"ha{t}", bufs=2)
        nc.tensor.matmul(
            ha_ps[:],
            x_sb[:, ts(t, 128)],
            w123t_sb[:],
            start=True,
            stop=not with_bias,
        )
        if with_bias:
            nc.tensor.matmul(ha_ps[:], ones_row[:], b123_sb[:], start=False, stop=True)
        scr = work.tile([128, 3 * NHID], f32, tag="scr1", name=f"scr1_{t}_{r}")
        nc.scalar.activation(scr[:], ha_ps[:], mybir.ActivationFunctionType.Square)
        q3 = small.tile([128, 3], f32, tag="q3", name=f"q3_{t}_{r}")
        nc.vector.reduce_sum(
            q3[:],
            scr[:].rearrange("p (k d) -> p k d", k=3),
            axis=mybir.AxisListType.X,
        )
        rn3 = _norm_scalars(nc, small, q3, "h1", width=3)
        nc.vector.tensor_mul(
            h1a[:, t, :].rearrange("p (k d) -> p k d", k=3),
            ha_ps[:].rearrange("p (k d) -> p k d", k=3),
            bcast_free(rn3[:], NHID),
        )
        # transpose into (d, n) layout: heads 0,1 in one 128x128 block
        tr_ps = ps_tile([128, 128], "tr", f"tr_{t}")
        nc.tensor.transpose(tr_ps[:], h1a[:, t, 0:128], id128[:])
        nc.vector.tensor_copy(h1t01[:, ts(t, 128)], tr_ps[:])
        tr2_ps = ps_tile([NHID, 128], "tr", f"tr2_{t}")
        nc.tensor.transpose(tr2_ps[:], h1a[:, t, 128:192], id128[:])
        nc.vector.tensor_copy(h1t2[:, ts(t, 128)], tr2_ps[:])
        # Gram accumulation: heads 0,1 in one 128-wide matmul
        nc.tensor.matmul(
            g01_ps[:],
            h1a[:, t, 0:128],
            h1a[:, t, 0:128],
            start=(t == 0),
            stop=(t == nt - 1),
        )
        nc.tensor.matmul(
            g2_ps[:],
            h1a[:, t, 128:192],
            h1a[:, t, 128:192],
            start=(t == 0),
            stop=(t == nt - 1),
        )

    # ---- global G as block-diag Gblk (heads 0,1) + g2 ----
    gblk = const.tile([128, 128], f32, name=f"gblk{r}")
    g2_sb = const.tile([NHID, NHID], f32, name=f"g2sb{r}")
    if 1 in PHASES:
        if use_collectives:
            gcat = const.tile([NHID, 3 * NHID], f32, name=f"gcat{r}")
            nc.vector.tensor_copy(gcat[:, 0:NHID], g01_ps[0:NHID, 0:NHID])
            gtmp = const.tile([128, NHID], f32, name=f"gtmp{r}")
            nc.vector.tensor_copy(gtmp[NHID:128, :], g01_ps[NHID:128, NHID:128])
            nc.sync.dma_start(out=gcat[:, NHID : 2 * NHID], in_=gtmp[NHID:128, :])
            nc.vector.tensor_copy(gcat[:, 2 * NHID :], g2_ps[:])
            g_out = _allreduce(nc, dram, gcat, [NHID, 3 * NHID], f"g_{r}")
            g_glob = const.tile([NHID, 3 * NHID], f32, name=f"g_glob{r}")
            nc.sync.dma_start(out=g_glob[:], in_=g_out[:])
            nc.vector.memset(gblk[:], 0.0)
            nc.vector.tensor_copy(gblk[0:NHID, 0:NHID], g_glob[:, 0:NHID])
            nc.sync.dma_start(
                out=gblk[NHID:128, NHID:128], in_=g_glob[:, NHID : 2 * NHID]
            )
            nc.vector.tensor_copy(g2_sb[:], g_glob[:, 2 * NHID :])
        else:
            nc.vector.memset(gblk[:], 0.0)
            nc.vector.tensor_copy(gblk[0:NHID, 0:NHID], g01_ps[0:NHID, 0:NHID])
            nc.vector.tensor_copy(gblk[NHID:128, NHID:128], g01_ps[NHID:128, NHID:128])
            nc.vector.tensor_copy(g2_sb[:], g2_ps[:])

    # ---- stage 1b: column norms of e, M ----
    m01_ps = ps_tile([128, 128], "acc01", "m01") if 2 in PHASES else None
    m2_ps = ps_tile([NHID, NHID], "acc2", "m2") if 2 in PHASES else None
    for t in range(nt) if 2 in PHASES else []:
        ta_ps = ps_tile([128, 3 * NHID], "mm128", f"ta_{t}", bufs=2)
        nc.tensor.matmul(
            ta_ps[:, 0:128],
            h1t01[:, ts(t, 128)],
            gblk[:],
            start=True,
            stop=True,
        )
        nc.tensor.matmul(
            ta_ps[:, 128:192],
            h1t2[:, ts(t, 128)],
            g2_sb[:],
            start=True,
            stop=True,
        )
        scr = work.tile([128, 3 * NHID], f32, tag="scr2", name=f"scr2_{t}_{r}")
        nc.vector.tensor_mul(scr[:], ta_ps[:], h1a[:, t, :])
        p3 = small.tile([128, 3], f32, tag="p3", name=f"p3_{t}_{r}")
        nc.vector.reduce_sum(
            p3[:],
            scr[:].rearrange("p (k d) -> p k d", k=3),
            axis=mybir.AxisListType.X,
        )
        icn3 = _norm_scalars(nc, small, p3, "cn", width=3)
        h1s = work.tile([128, 3 * NHID], f32, tag="h1s", name=f"h1s_{t}_{r}")
        nc.vector.tensor_mul(
            h1s[:].rearrange("p (k d) -> p k d", k=3),
            h1a[:, t, :].rearrange("p (k d) -> p k d", k=3),
            bcast_free(icn3[:], NHID),
        )
        nc.tensor.matmul(
            m01_ps[:],
            h1a[:, t, 0:128],
            h1s[:, 0:128],
            start=(t == 0),
            stop=(t == nt - 1),
        )
        nc.tensor.matmul(
            m2_ps[:],
            h1a[:, t, 128:192],
            h1s[:, 128:192],
            start=(t == 0),
            stop=(t == nt - 1),
        )

    # ---- global M as block-diag Mblk (heads 0,1) + m2 ----
    mblk = const.tile([128, 128], f32, name=f"mblk{r}")
    m2_sb = const.tile([NHID, NHID], f32, name=f"m2sb{r}")
    if 2 in PHASES:
        if use_collectives:
            mcat = const.tile([NHID, 3 * NHID], f32, name=f"mcat{r}")
            nc.vector.tensor_copy(mcat[:, 0:NHID], m01_ps[0:NHID, 0:NHID])
            mtmp = const.tile([128, NHID], f32, name=f"mtmp{r}")
            nc.vector.tensor_copy(mtmp[NHID:128, :], m01_ps[NHID:128, NHID:128])
            nc.sync.dma_start(out=mcat[:, NHID : 2 * NHID], in_=mtmp[NHID:128, :])
            nc.vector.tensor_copy(mcat[:, 2 * NHID :], m2_ps[:])
            m_out = _allreduce(nc, dram, mcat, [NHID, 3 * NHID], f"m_{r}")
            m_glob = const.tile([NHID, 3 * NHID], f32, name=f"m_glob{r}")
            nc.sync.dma_start(out=m_glob[:], in_=m_out[:])
            nc.vector.memset(mblk[:], 0.0)
            nc.vector.tensor_copy(mblk[0:NHID, 0:NHID], m_glob[:, 0:NHID])
            nc.sync.dma_start(
                out=mblk[NHID:128, NHID:128], in_=m_glob[:, NHID : 2 * NHID]
            )
            nc.vector.tensor_copy(m2_sb[:], m_glob[:, 2 * NHID :])
        else:
            nc.vector.memset(mblk[:], 0.0)
            nc.vector.tensor_copy(mblk[0:NHID, 0:NHID], m01_ps[0:NHID, 0:NHID])
            nc.vector.tensor_copy(mblk[NHID:128, NHID:128], m01_ps[NHID:128, NHID:128])
            nc.vector.tensor_copy(m2_sb[:], m2_ps[:])

    # ---- z = h1 @ M (transposed layout), elu -> hc ----
    def emit_elu(zt_ps, dst, parts, tag):
        # elu(z) = exp(min(z,0)) + (max(z,0) - 1)
        e_min = work.tile([parts, 512], f32, tag=f"emin{parts}", name=f"emin_{tag}_{r}")
        nc.vector.tensor_scalar_min(e_min[:], zt_ps[:], 0.0)
        e_exp = work.tile([parts, 512], f32, tag=f"eexp{parts}", name=f"eexp_{tag}_{r}")
        nc.scalar.activation(e_exp[:], e_min[:], mybir.ActivationFunctionType.Exp)
        e_max = work.tile([parts, 512], f32, tag=f"emax{parts}", name=f"emax_{tag}_{r}")
        nc.vector.tensor_scalar(
            out=e_max[:],
            in0=zt_ps[:],
            scalar1=0.0,
            scalar2=-1.0,
            op0=mybir.AluOpType.max,
            op1=mybir.AluOpType.add,
        )
        nc.vector.tensor_add(dst, e_exp[:], e_max[:])

    for c in range(nch) if 3 in PHASES else []:
        zt_ps = ps_tile([128, 512], "wide", f"zt01_{c}", bufs=2)
        nc.tensor.matmul(
            zt_ps[:], mblk[:], h1t01[:, ts(c, 512)], start=True, stop=True
        )
        emit_elu(zt_ps, hc_hi[:, ts(c, 512)], 128, f"01_{c}")
        zt2_ps = ps_tile([NHID, 512], "wide", f"zt2_{c}", bufs=2)
        nc.tensor.matmul(
            zt2_ps[:], m2_sb[:], h1t2[:, ts(c, 512)], start=True, stop=True
        )
        emit_elu(zt2_ps, hc_lo[:, ts(c, 512)], NHID, f"2_{c}")

    # ---- stage 2: output head (d = 16) ----
    h1oa = const.tile([128, nt, NCLASS], f32, name=f"h1oa{r}")
    h1ot = const.tile([NCLASS, nloc], f32, name=f"h1ot{r}")
    go_ps = ps_tile([NCLASS, NCLASS], "oacc", "go_ps") if 4 in PHASES else None
    for t in range(nt) if 4 in PHASES else []:
        hoa_ps = ps_tile([128, NCLASS], "mm128", f"hoa_{t}", bufs=2)
        nc.tensor.matmul(
            hoa_ps[:], hc_hi[:, ts(t, 128)], wot_hi_sb[:], start=True, stop=False
        )
        nc.tensor.matmul(
            hoa_ps[:],
            hc_lo[:, ts(t, 128)],
            wot_lo_sb[:],
            start=False,
            stop=not with_bias,
        )
        if with_bias:
            nc.tensor.matmul(hoa_ps[:], ones_row[:], bo_sb[:], start=False, stop=True)
        scr = work.tile([128, NCLASS], f32, tag="scro", name=f"scro_{t}_{r}")
        q = small.tile([128, 1], f32, tag="qo", name=f"qo_{t}_{r}")
        nc.scalar.activation(
            scr[:], hoa_ps[:], mybir.ActivationFunctionType.Square, accum_out=q[:]
        )
        rno = _norm_scalars(nc, small, q, "ho")
        nc.vector.tensor_scalar_mul(h1oa[:, t, :], hoa_ps[:], rno[:])
        tro_ps = ps_tile([NCLASS, 128], "tr", f"tro_{t}")
        nc.tensor.transpose(tro_ps[:], h1oa[:, t, :], id128[:])
        nc.vector.tensor_copy(h1ot[:, ts(t, 128)], tro_ps[:])
        nc.tensor.matmul(
            go_ps[:],
            h1oa[:, t, :],
            h1oa[:, t, :],
            start=(t == 0),
            stop=(t == nt - 1),
        )
    go_sb = const.tile([NCLASS, NCLASS], f32, name=f"go_sb{r}")
    if 4 in PHASES:
        nc.vector.tensor_copy(go_sb[:], go_ps[:])

    if use_collectives:
        go_out = _allreduce(nc, dram, go_sb, [NCLASS, NCLASS], f"go_{r}")
        go_glob = const.tile([NCLASS, NCLASS], f32, name=f"go_glob{r}")
        nc.sync.dma_start(out=go_glob[:], in_=go_out[:])
    else:
        go_glob = go_sb

    mo_ps = ps_tile([NCLASS, NCLASS], "oacc", "mo_ps") if 5 in PHASES else None
    for t in range(nt) if 5 in PHASES else []:
        to_ps = ps_tile([128, NCLASS], "mm128", f"to_{t}", bufs=2)
        nc.tensor.matmul(
            to_ps[:], h1ot[:, ts(t, 128)], go_glob[:], start=True, stop=True
        )
        scr = work.tile([128, NCLASS], f32, tag="scro2", name=f"scro2_{t}_{r}")
        p = small.tile([128, 1], f32, tag="po", name=f"po_{t}_{r}")
        nc.vector.tensor_mul(scr[:], to_ps[:], h1oa[:, t, :])
        nc.vector.reduce_sum(p[:], scr[:], axis=mybir.AxisListType.X)
        icno = _norm_scalars(nc, small, p, "cno")
        h1so = work.tile([128, NCLASS], f32, tag="h1so", name=f"h1so_{t}_{r}")
        nc.vector.tensor_scalar_mul(h1so[:], h1oa[:, t, :], icno[:])
        nc.tensor.matmul(
            mo_ps[:],
            h1oa[:, t, :],
            h1so[:],
            start=(t == 0),
            stop=(t == nt - 1),
        )
    mo_sb = const.tile([NCLASS, NCLASS], f32, name=f"mo_sb{r}")
    if 5 in PHASES:
        nc.vector.tensor_copy(mo_sb[:], mo_ps[:])

    if use_collectives:
        mo_out = _allreduce(nc, dram, mo_sb, [NCLASS, NCLASS], f"mo_{r}")
        mo_glob = const.tile([NCLASS, NCLASS], f32, name=f"mo_glob{r}")
        nc.sync.dma_start(out=mo_glob[:], in_=mo_out[:])
    else:
        mo_glob = mo_sb

    # ---- final: out = (h1o @ Mo).T = Mo.T @ h1o.T, no activation ----
    fot_sb = const.tile([NCLASS, nloc], f32, name=f"fot_sb{r}")
    for c in range(nch) if 6 in PHASES else []:
        fot_ps = ps_tile([NCLASS, 512], "wide", f"fot_{c}", bufs=2)
        nc.tensor.matmul(
            fot_ps[:], mo_glob[:], h1ot[:, ts(c, 512)], start=True, stop=True
        )
        nc.vector.tensor_copy(fot_sb[:, ts(c, 512)], fot_ps[:])
    if 6 in PHASES:
        nc.sync.dma_start(out=out_d[:], in_=fot_sb[:])


def build_program(reps=1, mode="rep", with_bias=False, loop=1):
    """Build the Bass program (shared by kernel() and test timing).

    loop > 1 wraps the body in an on-device For_i (timing amplification;
    only valid without collectives, i.e. mode="rep")."""
    key = (reps, mode, with_bias, loop, tuple(sorted(PHASES)))
    if key in _prog_cache:
        return _prog_cache[key]
    assert loop == 1 or mode == "rep", "device loop requires no collectives"

    _patch_tile_drain()
    import concourse.bass as bass
    import concourse.tile as tile
    import concourse.mybir as mybir
    from contextlib import ExitStack

    nloc = NLOC if mode == "shard" else N
    use_collectives = mode == "shard"

    f32 = mybir.dt.float32
    nc = bass.Bass(num_devices=N_CORES)
    tensors = {
        "xloc": nc.dram_tensor("xloc", [128, nloc], f32, kind="ExternalInput"),
        "w123t": nc.dram_tensor("w123t", [128, 3 * NHID], f32, kind="ExternalInput"),
        "b123": nc.dram_tensor("b123", [1, 3 * NHID], f32, kind="ExternalInput"),
        "wot_hi": nc.dram_tensor("wot_hi", [128, NCLASS], f32, kind="ExternalInput"),
        "wot_lo": nc.dram_tensor("wot_lo", [64, NCLASS], f32, kind="ExternalInput"),
        "bo": nc.dram_tensor("bo", [1, NCLASS], f32, kind="ExternalInput"),
        "outt": nc.dram_tensor("outt", [NCLASS, nloc], f32, kind="ExternalOutput"),
    }

    with tile.TileContext(nc) as tc:
        if loop > 1:
            with tc.For_i(0, loop, 1):
                for r in range(reps):
                    with ExitStack() as ctx:
                        _emit_body(
                            nc, tc, ctx, tensors, r, nloc, use_collectives, with_bias
                        )
        else:
            for r in range(reps):
                with ExitStack() as ctx:
                    _emit_body(
                        nc, tc, ctx, tensors, r, nloc, use_collectives, with_bias
                    )

    _split_multi_waits(nc)
    _prog_cache[key] = nc
    return nc


def make_in_maps(x, W1, b1, W2, b2, W3, b3, Wo, bo, mode="rep"):
    x_mem = np.asarray(x, dtype=np.float32).reshape(NFEAT, N)
    w123t = np.ascontiguousarray(
        np.concatenate(
            [np.asarray(W1).T, np.asarray(W2).T, np.asarray(W3).T], axis=1
        ),
        dtype=np.float32,
    )
    b123 = (
        np.concatenate([np.asarray(b1), np.asarray(b2), np.asarray(b3)])
        .reshape(1, 3 * NHID)
        .astype(np.float32)
    )
    wot = np.ascontiguousarray(np.asarray(Wo).T, dtype=np.float32)  # (192, 16)
    wot_hi = np.ascontiguousarray(wot[:128])
    wot_lo = np.ascontiguousarray(wot[128:])
    bo_r = np.asarray(bo).reshape(1, NCLASS).astype(np.float32)
    common = {
        "w123t": w123t,
        "b123": b123,
        "wot_hi": wot_hi,
        "wot_lo": wot_lo,
        "bo": bo_r,
    }
    in_maps = []
    for c in range(N_CORES):
        if mode == "shard":
            xc = np.ascontiguousarray(x_mem[:, c * NLOC : (c + 1) * NLOC])
        else:
            xc = x_mem
        in_maps.append({"xloc": xc, **common})
    return in_maps


def assemble_output(results, mode="rep"):
    if mode == "shard":
        slabs = [results[c]["outt"] for c in range(N_CORES)]
        full = np.concatenate(slabs, axis=1)  # (16, 4096)
    else:
        full = results[0]["outt"]
    return np.ascontiguousarray(full.reshape(1, NCLASS, 64, 64), dtype=np.float32)


def kernel(x, W1, b1, W2, b2, W3, b3, Wo, bo):
    from concourse.bass_utils import run_bass_kernel_spmd

    mode = "rep"
    with_bias = any(
        np.any(np.asarray(b)) for b in (b1, b2, b3, bo)
    )
    nc = build_program(reps=1, mode=mode, with_bias=with_bias)
    in_maps = make_in_maps(x, W1, b1, W2, b2, W3, b3, Wo, bo, mode=mode)
    res = run_bass_kernel_spmd(nc, in_maps, list(range(N_CORES)))
    return assemble_output(res.results, mode=mode)
